# revision 18
# baseline (speedup 1.0000x reference)
"""Trainium2 Bass kernel for nn_LinearCoeffGNN: coeffs = F0 @ N @ F0^T.

Math (approximate factorization of the reference, validated to ~3e-3
rel err vs the 2e-2 gate):
  - Q/K/V are rank-1 in x, so the attention block collapses (see the
    exact factorization): coeffs[b] = F0 N F0^T with F0 = [qv_h|qb_h|1]
    (P x 17) and N (17x17) data-dependent only through per-head scalars
    S1_h = sum_m s_hm, S2_h = sum_m s_hm^2, where s_hm is the
    softmax-tilted mean of x at tilt a_hm = (w_k . w_mem^T)[h,m].
  - The 17 columns of F0 are scalar functions of x_p alone; they are
    fit on the host with a 64-atom softplus basis and evaluated on
    device with ONE scalar-engine instruction + one matmul.
  - s(a) = M1(a)/M0(a) with M_j(a) = sum_p x_p^j e^{a x_p} analytic in
    a: Taylor via x-moments (k<=17), evaluated at 32 Chebyshev nodes in
    a; S1/S2 are Lagrange-weighted node sums (host-precomputed weights).
  - Final product: per 128-row chunk, PE matmul (17-contraction, f32r)
    -> PSUM -> DVE/ACT copy to bf16 -> DMA out.
Sharding: data-parallel over batch B=32 -> 4 batches/core on 8 cores.
Batches are stacked vertically at partitions {0,32,64,96} so the
[17, *] PSUM evacuations amortize across all four batches.
"""
import math

import numpy as np

import concourse.bacc as bacc
import concourse.bass as bass
import concourse.mybir as mybir
import concourse.tile as tile
from concourse import bass_utils

B, P = 32, 1024
HID, H, D = 512, 8, 64
MEM, RANK = 64, 64
NCORES = 8
BPC = B // NCORES          # batches per core
KB = 64                    # basis size (NE exp atoms + NE relu atoms)
NE = 32                    # exp atoms
KM = 12                    # moment Taylor order (moments 0..KM+1)
NG = 32                    # Chebyshev nodes for s(a)
NMOM = KM + 2              # 18 moment rows
BW = 304                   # const-blob free width

F32 = mybir.dt.float32
F32R = mybir.dt.float32r
BF16 = mybir.dt.bfloat16
AF = mybir.ActivationFunctionType
ALU = mybir.AluOpType

_CACHE = {}
TRACE = False


def _build():
    nc = bacc.Bacc("TRN2", target_bir_lowering=False, debug=False,
                   num_devices=NCORES)
    xs = nc.dram_tensor("xs", [BPC, P], F32, kind="ExternalInput").ap()
    blob = nc.dram_tensor("blob", [128, BW], F32, kind="ExternalInput").ap()
    out = nc.dram_tensor("out", [BPC, P, P], BF16, kind="ExternalOutput").ap()

    with tile.TileContext(nc) as tc:
        with tc.tile_pool(name="consts", bufs=1) as cp, \
             tc.tile_pool(name="work", bufs=2) as wp, \
             tc.tile_pool(name="stage", bufs=3) as sp, \
             tc.tile_pool(name="ps_small", bufs=2, space="PSUM") as pss, \
             tc.tile_pool(name="ps_fz", bufs=2, space="PSUM") as psf, \
             tc.tile_pool(name="ps_cc", bufs=3, space="PSUM") as psc, \
             tc.tile_pool(name="ps_dummy", bufs=1, space="PSUM") as psd:

            # ---- one-shot input DMAs ----
            XB = wp.tile([KB, BPC * P], F32, tag="XB")
            nc.sync.dma_start(out=XB[:, 0:P], in_=bass.AP(
                tensor=xs.tensor, offset=xs.offset, ap=[[0, KB], [1, P]]))
            blob_sb = cp.tile([128, BW], F32, tag="blob")
            nc.sync.dma_start(out=blob_sb, in_=blob)
            for _b in range(1, BPC):
                nc.sync.dma_start(
                    out=XB[:, _b * P:(_b + 1) * P],
                    in_=bass.AP(tensor=xs.tensor,
                                offset=xs.offset + _b * P,
                                ap=[[0, KB], [1, P]]))

            # const views into the blob
            xc_sb = blob_sb[:, 0:32]
            spab_sb = blob_sb[0:KB, 32:34]
            coefT_f = blob_sb[0:KB, 34:51]
            ctT_sb = blob_sb[0:17, 51:115]
            vand_f = blob_sb[0:NMOM, 115:179]
            wperm_sb = blob_sb[0:2 * NG, 179:228]
            mA_sb = blob_sb[0:17, 228:245]
            mB_sb = blob_sb[0:17, 245:262]
            cT_sb = blob_sb[0:17, 262:279]
            mT_sb = blob_sb[0:17, 279:296]
            id_sb = blob_sb[0:BPC, 296:300]
            ones128 = blob_sb[:, 300:301]

            coefT_sb = cp.tile([KB, 17], F32R, tag="coefT")
            nc.vector.tensor_copy(coefT_sb, coefT_f)
            vand_sb = cp.tile([NMOM, 2 * NG], F32R, tag="vand")
            nc.vector.tensor_copy(vand_sb, vand_f)

            # ---- basis (per batch, emitted first for ACT priority) ----
            bas = wp.tile([KB, BPC * P], F32R, tag="bas")
            for _b in range(BPC):
                bcol = slice(_b * P, (_b + 1) * P)
                nc.scalar.activation(bas[0:NE, bcol], XB[0:NE, bcol], AF.Exp,
                                     bias=spab_sb[0:NE, 1:2],
                                     scale=spab_sb[0:NE, 0:1])
                nc.scalar.activation(bas[NE:KB, bcol], XB[NE:KB, bcol],
                                     AF.Relu,
                                     bias=spab_sb[NE:KB, 1:2],
                                     scale=spab_sb[NE:KB, 0:1])

            # ---- moments: powers of x in chunk layout ----
            Pw = wp.tile([128, NMOM * 32], F32, tag="Pw")
            nc.vector.memset(Pw[:, 0:32], 1.0)
            nc.vector.tensor_copy(Pw[:, 32:64], xc_sb)
            for k in range(2, NMOM):
                nc.vector.tensor_mul(Pw[:, 32 * k:32 * (k + 1)],
                                     Pw[:, 32 * (k - 1):32 * k], xc_sb)
            Pred = wp.tile([128, NMOM * BPC], F32, tag="Pred")
            nc.vector.reduce_sum(
                Pred, Pw.rearrange("p (k b c) -> p (k b) c", k=NMOM, b=BPC),
                axis=mybir.AxisListType.X)
            m_all = wp.tile([NMOM, BPC], F32R, tag="m_all")
            predv = Pred.rearrange("p (k b) -> p k b", k=NMOM)
            for b in range(BPC):
                mcol = pss.tile([NMOM, 1], F32, tag="small")
                nc.tensor.matmul(mcol, predv[:, :, b], ones128,
                                 start=True, stop=True)
                nc.vector.tensor_copy(m_all[:, b:b + 1], mcol)

            # ---- s at Chebyshev nodes; S1/S2 -> ab vector ----
            M_ps = pss.tile([BPC, 2 * NG], F32, tag="small")
            nc.tensor.matmul(M_ps, m_all, vand_sb, start=True, stop=True)
            rec = wp.tile([BPC, NG], F32, tag="rec")
            nc.vector.reciprocal(rec, M_ps[:, 0:NG])
            s_sb = wp.tile([BPC, 2 * NG], F32, tag="s_sb")
            nc.vector.tensor_mul(s_sb[:, 0:NG], M_ps[:, NG:2 * NG], rec)
            nc.vector.tensor_mul(s_sb[:, NG:2 * NG], s_sb[:, 0:NG],
                                 s_sb[:, 0:NG])
            scat_ps = pss.tile([2 * NG, BPC], F32, tag="small")
            nc.tensor.transpose(scat_ps, s_sb, id_sb)
            scat_sb = wp.tile([2 * NG, BPC], F32, tag="scat")
            nc.vector.tensor_copy(scat_sb, scat_ps)
            ab_ps = pss.tile([49, BPC], F32, tag="small")
            nc.tensor.matmul(ab_ps, wperm_sb, scat_sb, start=True, stop=True)

            # ---- per-batch N and CN = C @ N ----
            cn_sbs = []
            for b in range(BPC):
                t1 = wp.tile([17, 17], F32, tag=f"t1_{b}")
                nc.vector.scalar_tensor_tensor(
                    t1, mA_sb, ab_ps[0:17, b:b + 1], cT_sb,
                    op0=ALU.mult, op1=ALU.add)
                tp = wp.tile([17, 17], F32, tag=f"tp_{b}")
                nc.vector.scalar_tensor_tensor(
                    tp, mB_sb, ab_ps[32:49, b:b + 1], t1,
                    op0=ALU.mult, op1=ALU.add)
                p1_ps = pss.tile([17, 17], F32, tag="small")
                nc.tensor.matmul(p1_ps, mT_sb, tp, start=True, stop=True)
                p1_sb = wp.tile([17, 17], F32, tag=f"p1_{b}")
                nc.vector.tensor_copy(p1_sb, p1_ps)
                n_ps = pss.tile([17, 17], F32, tag="small")
                nc.tensor.matmul(n_ps, tp, p1_sb, start=True, stop=True)
                n_sb = wp.tile([17, 17], F32, tag=f"n_{b}")
                nc.vector.tensor_copy(n_sb, n_ps)
                cn_ps = pss.tile([KB, 17], F32, tag="small")
                nc.tensor.matmul(cn_ps, ctT_sb, n_sb, start=True, stop=True)
                cn_sb = wp.tile([KB, 17], F32R, tag=f"cn_{b}")
                nc.vector.tensor_copy(cn_sb, cn_ps)
                cn_sbs.append(cn_sb)

            # ---- basis (per batch pair), then per-batch f0/z + output ----
            f0ts = [cp.tile([17, P], BF16, tag=f"f0t{i}", name=f"f0t{i}")
                    for i in range(BPC)]
            z_alls = [cp.tile([17, P], BF16, tag=f"z{i}", name=f"z{i}")
                      for i in range(BPC)]
            dummy_ps = psd.tile([17, 512], F32, tag="dummy")
            if True:
                for b in range(BPC):
                    for half in range(2):
                        fz = psf.tile([17, 512], F32, tag="fz")
                        nc.tensor.matmul(
                            fz, coefT_sb,
                            bas[:, b * P + half * 512:
                                b * P + (half + 1) * 512],
                            start=True, stop=True)
                        nc.any.tensor_copy(
                            f0ts[b][:, half * 512:(half + 1) * 512], fz)
                    for half in range(2):
                        fz = psf.tile([17, 512], F32, tag="fz")
                        nc.tensor.matmul(
                            fz, cn_sbs[b],
                            bas[:, b * P + half * 512:
                                b * P + (half + 1) * 512],
                            start=True, stop=True)
                        nc.any.tensor_copy(
                            z_alls[b][:, half * 512:(half + 1) * 512], fz)
                    # final product: 8 row chunks, staged 2 chunks per DMA.
                    # A filler matmul after each chunk keeps the PE array
                    # streaming through copy-waits so it holds max p-state.
                    zb = z_alls[b]
                    fb = f0ts[b]
                    for rcp in range(4):
                        st = sp.tile([128, 2 * P], BF16, tag="st")
                        for sub in range(2):
                            rc = 2 * rcp + sub
                            dst = st[:, sub * P:(sub + 1) * P]
                            for hf in range(2):
                                cc = psc.tile([128, 512], F32, tag="cc")
                                nc.tensor.matmul(
                                    cc, zb[:, rc * 128:(rc + 1) * 128],
                                    fb[:, hf * 512:(hf + 1) * 512],
                                    start=True, stop=True)
                                nc.any.tensor_copy(
                                    dst[:, hf * 512:(hf + 1) * 512], cc)
                            nc.tensor.matmul(dummy_ps, coefT_sb,
                                             bas[:, 0:512],
                                             start=True, stop=True)
                        nc.sync.dma_start(
                            out=bass.AP(
                                tensor=out.tensor,
                                offset=out.offset + b * P * P
                                + rcp * 256 * P,
                                ap=[[P, 128], [128 * P, 2], [1, P]]),
                            in_=st)
    nc.compile()
    return nc


def _softplus(u):
    return np.log1p(np.exp(-np.abs(u))) + np.maximum(u, 0.0)


def _phi(u):
    return np.where(u < 0, np.exp(np.minimum(u, 0.0)), u + 1.0)


def _host_consts(w_q, b_q, w_k, b_k, w_v, b_v, w_mem, w_u, b_u, w_v2, b_v2):
    w_q = w_q.astype(np.float64); b_q = b_q.astype(np.float64)
    w_v = w_v.astype(np.float64); b_v = b_v.astype(np.float64)
    # --- exp+relu basis fit of the 17 feature functions ---
    # atom j: func(scale_j * t + bias_j); rows 0..NE-1 exp, NE..KB-1 relu
    a_exp = np.linspace(-2.2, 2.2, NE)
    knots = np.linspace(-4.8, 4.8, NE // 2)
    AB = np.zeros((KB, 2))
    AB[0:NE, 0] = a_exp
    for i, k in enumerate(knots):
        AB[NE + 2 * i] = (1.0, -k)
        AB[NE + 2 * i + 1] = (-1.0, k)
    tg = np.linspace(-5.5, 5.5, 3001)
    u = tg[:, None] * AB[None, :, 0] + AB[None, :, 1]
    Bg = np.concatenate([np.exp(np.minimum(u[:, 0:NE], 30.0)),
                         np.maximum(u[:, NE:KB], 0.0)], axis=1)
    targ = np.zeros((len(tg), 17))
    for h in range(H):
        sl = slice(h * D, (h + 1) * D)
        ph = _phi(tg[:, None] * w_q[sl][None, :] + b_q[sl][None, :])
        targ[:, 2 * h] = ph @ w_v[sl]
        targ[:, 2 * h + 1] = ph @ b_v[sl]
    targ[:, 16] = 1.0
    sc = np.linalg.norm(Bg, axis=0)
    Bn = Bg / sc
    C = np.linalg.solve(Bn.T @ Bn + 1e-7 * np.eye(KB), Bn.T @ targ)
    C = (C.T / sc).T                                    # (KB, 17)
    Cp = C

    # --- Chebyshev nodes in a, Taylor-moment Vandermonde, S1/S2 weights ---
    A = (w_k.reshape(H, D).astype(np.float64) @ w_mem.T.astype(np.float64))
    a_flat = A.reshape(-1)                              # (512,) h-major
    amax = np.abs(a_flat).max() * 1.0001
    g = np.arange(NG)
    nodes = amax * np.cos(np.pi * (g + 0.5) / NG)
    ks = np.arange(KM + 1)
    fact = np.array([math.factorial(k) for k in ks])
    vand = np.zeros((NMOM, 2 * NG))
    vand[0:KM + 1, 0:NG] = nodes[None, :] ** ks[:, None] / fact[:, None]
    vand[1:KM + 2, NG:2 * NG] = vand[0:KM + 1, 0:NG]
    # Lagrange (via Chebyshev-Vandermonde) interpolation weights
    Tn = np.polynomial.chebyshev.chebvander(nodes / amax, NG - 1)   # (NG, NG)
    Ta = np.polynomial.chebyshev.chebvander(a_flat / amax, NG - 1)  # (512,NG)
    L = Ta @ np.linalg.inv(Tn)                          # (512, NG)
    W1 = np.zeros((NG, H))
    for h in range(H):
        W1[:, h] = L[h * MEM:(h + 1) * MEM].sum(0)
    Wcat = np.zeros((2 * NG, 16))
    Wcat[0:NG, 0:8] = W1                                # S1
    Wcat[NG:2 * NG, 8:16] = W1                          # S2
    # scol->ab permutation (same convention as the exact factorization)
    perm = np.zeros((16, 49))
    for h in range(H):
        perm[8 + h, 2 * h] = 1.0                        # a_vec[2h] = S2_h
        perm[h, 32 + 2 * h] = 1.0                       # b_vec[2h] = S1_h
        perm[h, 32 + 2 * h + 1] = 1.0
    Wperm = Wcat @ perm                                 # (2*NG, 49)

    # --- N-machinery masks and M' ---
    Gu = np.zeros((17, RANK)); Gv = np.zeros((17, RANK))
    for h in range(H):
        sl = slice(h * D, (h + 1) * D)
        Gu[2 * h] = w_u[:, sl].astype(np.float64) @ w_v[sl]
        Gu[2 * h + 1] = w_u[:, sl].astype(np.float64) @ b_v[sl]
        Gv[2 * h] = w_v2[:, sl].astype(np.float64) @ w_v[sl]
        Gv[2 * h + 1] = w_v2[:, sl].astype(np.float64) @ b_v[sl]
    Gu[16] = b_u; Gv[16] = b_v2
    Mp = Gu @ Gv.T
    mA = np.zeros((17, 17)); mB = np.zeros((17, 17)); cT = np.zeros((17, 17))
    for h in range(H):
        mA[2 * h, 2 * h] = 1.0
        mB[2 * h, 2 * h + 1] = 1.0
        mB[2 * h + 1, 2 * h] = 1.0
        cT[2 * h + 1, 2 * h + 1] = float(MEM)
    cT[16, 16] = 1.0
    f32 = lambda x: np.ascontiguousarray(x, np.float32)
    blob = np.zeros((128, BW), np.float32)
    blob[0:KB, 32:34] = AB
    blob[0:KB, 34:51] = C
    blob[0:17, 51:115] = C.T
    blob[0:NMOM, 115:179] = vand
    blob[0:2 * NG, 179:228] = Wperm
    blob[0:17, 228:245] = mA
    blob[0:17, 245:262] = mB
    blob[0:17, 262:279] = cT
    blob[0:17, 279:296] = Mp.T
    blob[0:BPC, 296:300] = np.eye(BPC)
    blob[:, 300] = 1.0
    return f32(blob)


def kernel(**inputs):
    x = np.ascontiguousarray(inputs["x"], dtype=np.float32)
    blob = _host_consts(
        *(np.asarray(inputs[k], np.float32) for k in
          ["w_q", "b_q", "w_k", "b_k", "w_v", "b_v", "w_mem",
           "w_u", "b_u", "w_v2", "b_v2"]))
    if "nc" not in _CACHE:
        _CACHE["nc"] = _build()
    nc = _CACHE["nc"]
    in_maps = []
    for c in range(NCORES):
        xsl = x[c * BPC:(c + 1) * BPC]                   # (BPC, P)
        cb = blob.copy()
        # chunk layout: xc[pp, b*8+cc] = x[b, cc*128+pp]
        cb[:, 0:32] = xsl.reshape(BPC, 8, 128).transpose(2, 0, 1).reshape(
            128, BPC * 8)
        in_maps.append({"xs": xsl.copy(), "blob": cb})
    res = bass_utils.run_bass_kernel_spmd(
        nc, in_maps, core_ids=list(range(NCORES)), trace=TRACE)
    _CACHE["last_res"] = res
    outs = [np.asarray(res.results[c]["out"]).astype(np.float32)
            for c in range(NCORES)]
    return np.concatenate(outs, 0)


# revision 19
# speedup vs baseline: 1.2307x; 1.2307x over previous
"""Trainium2 Bass kernel for nn_LinearCoeffGNN: coeffs = F0 @ N @ F0^T.

Math (approximate factorization of the reference, validated to ~3e-3
rel err vs the 2e-2 gate):
  - Q/K/V are rank-1 in x, so the attention block collapses (see the
    exact factorization): coeffs[b] = F0 N F0^T with F0 = [qv_h|qb_h|1]
    (P x 17) and N (17x17) data-dependent only through per-head scalars
    S1_h = sum_m s_hm, S2_h = sum_m s_hm^2, where s_hm is the
    softmax-tilted mean of x at tilt a_hm = (w_k . w_mem^T)[h,m].
  - The 17 columns of F0 are scalar functions of x_p alone; they are
    fit on the host with a 64-atom softplus basis and evaluated on
    device with ONE scalar-engine instruction + one matmul.
  - s(a) = M1(a)/M0(a) with M_j(a) = sum_p x_p^j e^{a x_p} analytic in
    a: Taylor via x-moments (k<=17), evaluated at 32 Chebyshev nodes in
    a; S1/S2 are Lagrange-weighted node sums (host-precomputed weights).
  - Final product: per 128-row chunk, PE matmul (17-contraction, f32r)
    -> PSUM -> DVE/ACT copy to bf16 -> DMA out.
Sharding: data-parallel over batch B=32 -> 4 batches/core on 8 cores.
Batches are stacked vertically at partitions {0,32,64,96} so the
[17, *] PSUM evacuations amortize across all four batches.
"""
import math

import numpy as np

import concourse.bacc as bacc
import concourse.bass as bass
import concourse.mybir as mybir
import concourse.tile as tile
from concourse import bass_utils

B, P = 32, 1024
HID, H, D = 512, 8, 64
MEM, RANK = 64, 64
NCORES = 8
BPC = B // NCORES          # batches per core
KB = 64                    # basis size (NE exp atoms + NE relu atoms)
NE = 32                    # exp atoms
KM = 12                    # moment Taylor order (moments 0..KM+1)
NG = 32                    # Chebyshev nodes for s(a)
NMOM = KM + 2              # 18 moment rows
BW = 304                   # const-blob free width

F32 = mybir.dt.float32
F32R = mybir.dt.float32r
BF16 = mybir.dt.bfloat16
AF = mybir.ActivationFunctionType
ALU = mybir.AluOpType

_CACHE = {}
TRACE = False


def _build():
    nc = bacc.Bacc("TRN2", target_bir_lowering=False, debug=False,
                   num_devices=NCORES)
    xs = nc.dram_tensor("xs", [BPC, P], F32, kind="ExternalInput").ap()
    blob = nc.dram_tensor("blob", [128, BW], F32, kind="ExternalInput").ap()
    out = nc.dram_tensor("out", [BPC, P, P], BF16, kind="ExternalOutput").ap()

    with tile.TileContext(nc) as tc:
        with tc.tile_pool(name="consts", bufs=1) as cp, \
             tc.tile_pool(name="work", bufs=2) as wp, \
             tc.tile_pool(name="stage", bufs=3) as sp, \
             tc.tile_pool(name="ps_small", bufs=2, space="PSUM") as pss, \
             tc.tile_pool(name="ps_fz", bufs=2, space="PSUM") as psf, \
             tc.tile_pool(name="ps_cc", bufs=4, space="PSUM") as psc:

            # ---- one-shot input DMAs ----
            XB = wp.tile([KB, BPC * P], F32, tag="XB")
            nc.sync.dma_start(out=XB[:, 0:P], in_=bass.AP(
                tensor=xs.tensor, offset=xs.offset, ap=[[0, KB], [1, P]]))
            blob_sb = cp.tile([128, BW], F32, tag="blob")
            nc.sync.dma_start(out=blob_sb, in_=blob)
            for _b in range(1, BPC):
                nc.sync.dma_start(
                    out=XB[:, _b * P:(_b + 1) * P],
                    in_=bass.AP(tensor=xs.tensor,
                                offset=xs.offset + _b * P,
                                ap=[[0, KB], [1, P]]))

            # const views into the blob
            xc_sb = blob_sb[:, 0:32]
            spab_sb = blob_sb[0:KB, 32:34]
            coefT_f = blob_sb[0:KB, 34:51]
            ctT_sb = blob_sb[0:17, 51:115]
            vand_f = blob_sb[0:NMOM, 115:179]
            wperm_sb = blob_sb[0:2 * NG, 179:228]
            mA_sb = blob_sb[0:17, 228:245]
            mB_sb = blob_sb[0:17, 245:262]
            cT_sb = blob_sb[0:17, 262:279]
            mT_sb = blob_sb[0:17, 279:296]
            id_sb = blob_sb[0:BPC, 296:300]
            ones128 = blob_sb[:, 300:301]

            coefT_sb = cp.tile([KB, 17], F32R, tag="coefT")
            nc.vector.tensor_copy(coefT_sb, coefT_f)
            vand_sb = cp.tile([NMOM, 2 * NG], F32R, tag="vand")
            nc.vector.tensor_copy(vand_sb, vand_f)

            # ---- basis (per batch, emitted first for ACT priority) ----
            bas = wp.tile([KB, BPC * P], F32R, tag="bas")
            for _b in range(BPC):
                bcol = slice(_b * P, (_b + 1) * P)
                nc.scalar.activation(bas[0:NE, bcol], XB[0:NE, bcol], AF.Exp,
                                     bias=spab_sb[0:NE, 1:2],
                                     scale=spab_sb[0:NE, 0:1])
                nc.scalar.activation(bas[NE:KB, bcol], XB[NE:KB, bcol],
                                     AF.Relu,
                                     bias=spab_sb[NE:KB, 1:2],
                                     scale=spab_sb[NE:KB, 0:1])

            # ---- moments: powers of x in chunk layout ----
            Pw = wp.tile([128, NMOM * 32], F32, tag="Pw")
            nc.vector.memset(Pw[:, 0:32], 1.0)
            nc.vector.tensor_copy(Pw[:, 32:64], xc_sb)
            for k in range(2, NMOM):
                nc.vector.tensor_mul(Pw[:, 32 * k:32 * (k + 1)],
                                     Pw[:, 32 * (k - 1):32 * k], xc_sb)
            Pred = wp.tile([128, NMOM * BPC], F32, tag="Pred")
            nc.vector.reduce_sum(
                Pred, Pw.rearrange("p (k b c) -> p (k b) c", k=NMOM, b=BPC),
                axis=mybir.AxisListType.X)
            m_all = wp.tile([NMOM, BPC], F32R, tag="m_all")
            predv = Pred.rearrange("p (k b) -> p k b", k=NMOM)
            for b in range(BPC):
                mcol = pss.tile([NMOM, 1], F32, tag="small")
                nc.tensor.matmul(mcol, predv[:, :, b], ones128,
                                 start=True, stop=True)
                nc.vector.tensor_copy(m_all[:, b:b + 1], mcol)

            # ---- s at Chebyshev nodes; S1/S2 -> ab vector ----
            M_ps = pss.tile([BPC, 2 * NG], F32, tag="small")
            nc.tensor.matmul(M_ps, m_all, vand_sb, start=True, stop=True)
            rec = wp.tile([BPC, NG], F32, tag="rec")
            nc.vector.reciprocal(rec, M_ps[:, 0:NG])
            s_sb = wp.tile([BPC, 2 * NG], F32, tag="s_sb")
            nc.vector.tensor_mul(s_sb[:, 0:NG], M_ps[:, NG:2 * NG], rec)
            nc.vector.tensor_mul(s_sb[:, NG:2 * NG], s_sb[:, 0:NG],
                                 s_sb[:, 0:NG])
            scat_ps = pss.tile([2 * NG, BPC], F32, tag="small")
            nc.tensor.transpose(scat_ps, s_sb, id_sb)
            scat_sb = wp.tile([2 * NG, BPC], F32, tag="scat")
            nc.vector.tensor_copy(scat_sb, scat_ps)
            ab_ps = pss.tile([49, BPC], F32, tag="small")
            nc.tensor.matmul(ab_ps, wperm_sb, scat_sb, start=True, stop=True)

            # ---- per-batch N and CN = C @ N ----
            cn_sbs = []
            for b in range(BPC):
                t1 = wp.tile([17, 17], F32, tag=f"t1_{b}")
                nc.vector.scalar_tensor_tensor(
                    t1, mA_sb, ab_ps[0:17, b:b + 1], cT_sb,
                    op0=ALU.mult, op1=ALU.add)
                tp = wp.tile([17, 17], F32, tag=f"tp_{b}")
                nc.vector.scalar_tensor_tensor(
                    tp, mB_sb, ab_ps[32:49, b:b + 1], t1,
                    op0=ALU.mult, op1=ALU.add)
                p1_ps = pss.tile([17, 17], F32, tag="small")
                nc.tensor.matmul(p1_ps, mT_sb, tp, start=True, stop=True)
                p1_sb = wp.tile([17, 17], F32, tag=f"p1_{b}")
                nc.vector.tensor_copy(p1_sb, p1_ps)
                n_ps = pss.tile([17, 17], F32, tag="small")
                nc.tensor.matmul(n_ps, tp, p1_sb, start=True, stop=True)
                n_sb = wp.tile([17, 17], F32, tag=f"n_{b}")
                nc.vector.tensor_copy(n_sb, n_ps)
                cn_ps = pss.tile([KB, 17], F32, tag="small")
                nc.tensor.matmul(cn_ps, ctT_sb, n_sb, start=True, stop=True)
                cn_sb = wp.tile([KB, 17], F32R, tag=f"cn_{b}")
                nc.vector.tensor_copy(cn_sb, cn_ps)
                cn_sbs.append(cn_sb)

            # ---- basis (per batch pair), then per-batch f0/z + output ----
            f0ts = [cp.tile([17, P], BF16, tag=f"f0t{i}", name=f"f0t{i}")
                    for i in range(BPC)]
            z_alls = [cp.tile([17, P], BF16, tag=f"z{i}", name=f"z{i}")
                      for i in range(BPC)]
            if True:
                for b in range(BPC):
                    for half in range(2):
                        fz = psf.tile([17, 512], F32, tag="fz")
                        nc.tensor.matmul(
                            fz, coefT_sb,
                            bas[:, b * P + half * 512:
                                b * P + (half + 1) * 512],
                            start=True, stop=True)
                        nc.any.tensor_copy(
                            f0ts[b][:, half * 512:(half + 1) * 512], fz)
                    for half in range(2):
                        fz = psf.tile([17, 512], F32, tag="fz")
                        nc.tensor.matmul(
                            fz, cn_sbs[b],
                            bas[:, b * P + half * 512:
                                b * P + (half + 1) * 512],
                            start=True, stop=True)
                        nc.any.tensor_copy(
                            z_alls[b][:, half * 512:(half + 1) * 512], fz)
                    # final product: 8 row chunks, staged 2 chunks per DMA.
                    # A filler matmul after each chunk keeps the PE array
                    # streaming through copy-waits so it holds max p-state.
                    zb = z_alls[b]
                    fb = f0ts[b]
                    for rcp in range(4):
                        st = sp.tile([128, 2 * P], BF16, tag="st")
                        for sub in range(2):
                            rc = 2 * rcp + sub
                            dst = st[:, sub * P:(sub + 1) * P]
                            for hf in range(2):
                                cc = psc.tile([128, 512], F32, tag="cc")
                                nc.tensor.matmul(
                                    cc, zb[:, rc * 128:(rc + 1) * 128],
                                    fb[:, hf * 512:(hf + 1) * 512],
                                    start=True, stop=True)
                                nc.any.tensor_copy(
                                    dst[:, hf * 512:(hf + 1) * 512], cc)
                        nc.sync.dma_start(
                            out=bass.AP(
                                tensor=out.tensor,
                                offset=out.offset + b * P * P
                                + rcp * 256 * P,
                                ap=[[P, 128], [128 * P, 2], [1, P]]),
                            in_=st)
    nc.compile()
    return nc


def _softplus(u):
    return np.log1p(np.exp(-np.abs(u))) + np.maximum(u, 0.0)


def _phi(u):
    return np.where(u < 0, np.exp(np.minimum(u, 0.0)), u + 1.0)


def _host_consts(w_q, b_q, w_k, b_k, w_v, b_v, w_mem, w_u, b_u, w_v2, b_v2):
    w_q = w_q.astype(np.float64); b_q = b_q.astype(np.float64)
    w_v = w_v.astype(np.float64); b_v = b_v.astype(np.float64)
    # --- exp+relu basis fit of the 17 feature functions ---
    # atom j: func(scale_j * t + bias_j); rows 0..NE-1 exp, NE..KB-1 relu
    a_exp = np.linspace(-2.2, 2.2, NE)
    knots = np.linspace(-4.8, 4.8, NE // 2)
    AB = np.zeros((KB, 2))
    AB[0:NE, 0] = a_exp
    for i, k in enumerate(knots):
        AB[NE + 2 * i] = (1.0, -k)
        AB[NE + 2 * i + 1] = (-1.0, k)
    tg = np.linspace(-5.5, 5.5, 3001)
    u = tg[:, None] * AB[None, :, 0] + AB[None, :, 1]
    Bg = np.concatenate([np.exp(np.minimum(u[:, 0:NE], 30.0)),
                         np.maximum(u[:, NE:KB], 0.0)], axis=1)
    targ = np.zeros((len(tg), 17))
    for h in range(H):
        sl = slice(h * D, (h + 1) * D)
        ph = _phi(tg[:, None] * w_q[sl][None, :] + b_q[sl][None, :])
        targ[:, 2 * h] = ph @ w_v[sl]
        targ[:, 2 * h + 1] = ph @ b_v[sl]
    targ[:, 16] = 1.0
    sc = np.linalg.norm(Bg, axis=0)
    Bn = Bg / sc
    C = np.linalg.solve(Bn.T @ Bn + 1e-7 * np.eye(KB), Bn.T @ targ)
    C = (C.T / sc).T                                    # (KB, 17)
    Cp = C

    # --- Chebyshev nodes in a, Taylor-moment Vandermonde, S1/S2 weights ---
    A = (w_k.reshape(H, D).astype(np.float64) @ w_mem.T.astype(np.float64))
    a_flat = A.reshape(-1)                              # (512,) h-major
    amax = np.abs(a_flat).max() * 1.0001
    g = np.arange(NG)
    nodes = amax * np.cos(np.pi * (g + 0.5) / NG)
    ks = np.arange(KM + 1)
    fact = np.array([math.factorial(k) for k in ks])
    vand = np.zeros((NMOM, 2 * NG))
    vand[0:KM + 1, 0:NG] = nodes[None, :] ** ks[:, None] / fact[:, None]
    vand[1:KM + 2, NG:2 * NG] = vand[0:KM + 1, 0:NG]
    # Lagrange (via Chebyshev-Vandermonde) interpolation weights
    Tn = np.polynomial.chebyshev.chebvander(nodes / amax, NG - 1)   # (NG, NG)
    Ta = np.polynomial.chebyshev.chebvander(a_flat / amax, NG - 1)  # (512,NG)
    L = Ta @ np.linalg.inv(Tn)                          # (512, NG)
    W1 = np.zeros((NG, H))
    for h in range(H):
        W1[:, h] = L[h * MEM:(h + 1) * MEM].sum(0)
    Wcat = np.zeros((2 * NG, 16))
    Wcat[0:NG, 0:8] = W1                                # S1
    Wcat[NG:2 * NG, 8:16] = W1                          # S2
    # scol->ab permutation (same convention as the exact factorization)
    perm = np.zeros((16, 49))
    for h in range(H):
        perm[8 + h, 2 * h] = 1.0                        # a_vec[2h] = S2_h
        perm[h, 32 + 2 * h] = 1.0                       # b_vec[2h] = S1_h
        perm[h, 32 + 2 * h + 1] = 1.0
    Wperm = Wcat @ perm                                 # (2*NG, 49)

    # --- N-machinery masks and M' ---
    Gu = np.zeros((17, RANK)); Gv = np.zeros((17, RANK))
    for h in range(H):
        sl = slice(h * D, (h + 1) * D)
        Gu[2 * h] = w_u[:, sl].astype(np.float64) @ w_v[sl]
        Gu[2 * h + 1] = w_u[:, sl].astype(np.float64) @ b_v[sl]
        Gv[2 * h] = w_v2[:, sl].astype(np.float64) @ w_v[sl]
        Gv[2 * h + 1] = w_v2[:, sl].astype(np.float64) @ b_v[sl]
    Gu[16] = b_u; Gv[16] = b_v2
    Mp = Gu @ Gv.T
    mA = np.zeros((17, 17)); mB = np.zeros((17, 17)); cT = np.zeros((17, 17))
    for h in range(H):
        mA[2 * h, 2 * h] = 1.0
        mB[2 * h, 2 * h + 1] = 1.0
        mB[2 * h + 1, 2 * h] = 1.0
        cT[2 * h + 1, 2 * h + 1] = float(MEM)
    cT[16, 16] = 1.0
    f32 = lambda x: np.ascontiguousarray(x, np.float32)
    blob = np.zeros((128, BW), np.float32)
    blob[0:KB, 32:34] = AB
    blob[0:KB, 34:51] = C
    blob[0:17, 51:115] = C.T
    blob[0:NMOM, 115:179] = vand
    blob[0:2 * NG, 179:228] = Wperm
    blob[0:17, 228:245] = mA
    blob[0:17, 245:262] = mB
    blob[0:17, 262:279] = cT
    blob[0:17, 279:296] = Mp.T
    blob[0:BPC, 296:300] = np.eye(BPC)
    blob[:, 300] = 1.0
    return f32(blob)


def kernel(**inputs):
    x = np.ascontiguousarray(inputs["x"], dtype=np.float32)
    blob = _host_consts(
        *(np.asarray(inputs[k], np.float32) for k in
          ["w_q", "b_q", "w_k", "b_k", "w_v", "b_v", "w_mem",
           "w_u", "b_u", "w_v2", "b_v2"]))
    if "nc" not in _CACHE:
        _CACHE["nc"] = _build()
    nc = _CACHE["nc"]
    in_maps = []
    for c in range(NCORES):
        xsl = x[c * BPC:(c + 1) * BPC]                   # (BPC, P)
        cb = blob.copy()
        # chunk layout: xc[pp, b*8+cc] = x[b, cc*128+pp]
        cb[:, 0:32] = xsl.reshape(BPC, 8, 128).transpose(2, 0, 1).reshape(
            128, BPC * 8)
        in_maps.append({"xs": xsl.copy(), "blob": cb})
    res = bass_utils.run_bass_kernel_spmd(
        nc, in_maps, core_ids=list(range(NCORES)), trace=TRACE)
    _CACHE["last_res"] = res
    outs = [np.asarray(res.results[c]["out"]).astype(np.float32)
            for c in range(NCORES)]
    return np.concatenate(outs, 0)
